# revision 42
# baseline (speedup 1.0000x reference)
"""Trainium2 Bass kernel for nn_CLIPVisionTower (latent-token attention block).

Strategy: data-parallel over batch (16 batches -> 8 cores x 2). Each core runs the
full block for its 2 batch elements; host concatenates outputs. No collectives.

Layout notes:
- All weights are passed host-transposed ([d_in, d_out]) and declared as float32r
  DRAM (raw fp32 bytes; the PE f32r path runs 1 cycle/row at free-dim >= 256).
- Activations flow "transposed" (feature dim on SBUF partitions): kvT holds
  [latt | hidden]^T per batch. Keys are zero-padded to 768 rows so every
  attention s-chunk is a full 128-row matmul; padded keys contribute exp(0)=1
  against V rows that are exactly zero and a ones-column that the host zeroes,
  so no masking instructions are needed.
- Softmax skips max-subtraction (logits*0.125 is O(5); exp is safe in fp32) and
  runs in key-major layout: exp on ScalarE straight out of PSUM; the Z row rides
  along as a 65th ones-column of V; 1/Z is computed by DVE reciprocal straight
  from PSUM and broadcast across partitions with a partition-stride-0 SWDGE DMA.
- Attention tensors (Q^T, K^T, V, exp(probs)) are bf16; projections are f32r.
"""

import sys

sys.path.insert(0, "/opt/trn_rl_repo")

import numpy as np
import ml_dtypes

import concourse.bass as bass
import concourse.mybir as mybir
import concourse.tile as tile
from concourse import bacc
from concourse.bass_utils import run_bass_kernel_spmd
from concourse.masks import make_identity

B, T, D = 16, 577, 1024
L, D_LLM = 64, 4096
H, HD = 16, 64
SCALE = HD ** -0.5
S = L + T            # 641 kv rows
SP = 768             # padded key rows for attention (6 * 128)
NC = 8               # cores
BPC = B // NC        # batches per core = 2

F32 = mybir.dt.float32
F32R = mybir.dt.float32r
BF16 = mybir.dt.bfloat16
Exp = mybir.ActivationFunctionType.Exp
Identity = mybir.ActivationFunctionType.Identity
MULT = mybir.AluOpType.mult

# f32r matmuls need even output widths
TQ = 578             # padded token axis for QT
SQ = 642             # padded kv column count (even)
TC2 = [(0, 290), (290, 578)]          # Q-proj psum chunks
KC_HI = (384, 642)                    # K-proj s-chunk independent of latt
KC_LO = (0, 384)                      # K-proj s-chunk that needs latt


def build_nc(zero_bias: bool):
    nc = bacc.Bacc(None, target_bir_lowering=False)

    kvT_d = nc.dram_tensor("kvT", [BPC, D, TQ], F32R, kind="ExternalInput")
    lrT_d = nc.dram_tensor("lrT", [D_LLM, BPC * L], BF16, kind="ExternalInput")
    WpT_d = nc.dram_tensor("WpT", [D_LLM, D], BF16, kind="ExternalInput")
    WqT_d = nc.dram_tensor("WqT", [D, D], F32R, kind="ExternalInput")
    WkT_d = nc.dram_tensor("WkT", [D, D], F32R, kind="ExternalInput")
    WvT_d = nc.dram_tensor("WvT", [D, D], F32R, kind="ExternalInput")
    WoT_d = nc.dram_tensor("WoT", [D, D], F32R, kind="ExternalInput")
    bq_d = nc.dram_tensor("bq2", [128, 8], F32, kind="ExternalInput")
    bk_d = nc.dram_tensor("bk2", [128, 8], F32, kind="ExternalInput")
    bp_d = nc.dram_tensor("bp2", [128, 8], F32, kind="ExternalInput")
    bv_d = nc.dram_tensor("bv2", [128, 8], F32, kind="ExternalInput")
    bo_d = nc.dram_tensor("bo2", [1, D], F32R, kind="ExternalInput")
    onesv_d = nc.dram_tensor("onesv", [128, 6, BPC, H], BF16, kind="ExternalInput")
    out_d = nc.dram_tensor("out2", [BPC, T, D], F32, kind="ExternalOutput")

    with tile.TileContext(nc) as tc:
        with tc.tile_pool(name="big", bufs=1) as big:
            # ---- persistent tensors ----
            QT = big.tile([128, 8, BPC, TQ], BF16, tag="qt")
            KT = big.tile([128, 8, BPC, SP], BF16, tag="kt")
            V = big.tile([128, 6, BPC, H * 65], BF16, tag="v")
            ctxT = big.tile([128, 8, BPC, T], F32R, tag="ctx")

            ident = big.tile([128, 128], BF16, tag="ident")
            bq_sb = big.tile([128, 8], F32, tag="bq")
            bk_sb = big.tile([128, 8], F32, tag="bk")
            bp_sb = big.tile([128, 8], F32, tag="bp")
            bv_sb = big.tile([128, 8], F32, tag="bv")

            with (
                tc.tile_pool(name="kvpool", bufs=1) as kvpool,
                tc.tile_pool(name="wpool", bufs=2) as wpool,
                tc.tile_pool(name="pp", bufs=3, space="PSUM") as pp,
            ):
                kv_sb = kvpool.tile([128, 8, BPC, SQ], F32R, tag="kv")
                kv_re = kvT_d[:, :, :].rearrange("b (j p) s -> p j b s", p=128)
                wq_re = WqT_d[:, :].rearrange("(k p) o -> p k o", p=128)
                wk_re = WkT_d[:, :].rearrange("(k p) o -> p k o", p=128)
                wv_re = WvT_d[:, :].rearrange("(k p) o -> p k o", p=128)
                wp_re = WpT_d[:, :].rearrange("(k p) o -> p k o", p=128)

                # ---- Q projection (emitted first: smallest DMA prefix) ----
                wq = [None, None]
                for oh in range(2):
                    wq[oh] = wpool.tile([128, 8, 512], F32R, tag="w", name=f"wq{oh}")
                    nc.sync.dma_start(wq[oh], wq_re[:, :, oh * 512:(oh + 1) * 512])
                    if oh == 0:
                        for b in range(BPC):
                            nc.sync.dma_start(
                                kv_sb[:, :, b, L:SQ], kv_re[:, :, b, :])
                        # consts ride behind the critical loads
                        make_identity(nc, ident)
                        nc.sync.dma_start(bq_sb, bq_d[:, :])
                        nc.sync.dma_start(bk_sb, bk_d[:, :])
                        nc.sync.dma_start(bp_sb, bp_d[:, :])
                        nc.sync.dma_start(bv_sb, bv_d[:, :])
                    for b in range(BPC):
                        for j4 in range(4):
                            j = oh * 4 + j4
                            for (t0, t1) in TC2:
                                w = t1 - t0
                                ps = pp.tile([128, 512], F32, tag="pp")
                                for k in range(8):
                                    nc.tensor.matmul(
                                        ps[:, :w],
                                        wq[oh][:, k, j4 * 128:(j4 + 1) * 128],
                                        kv_sb[:, k, b, L + t0:L + t1],
                                        start=(k == 0), stop=(k == 7),
                                    )
                                nc.scalar.activation(
                                    QT[:, j, b, t0:t1], ps[:, :w],
                                    Identity, bias=bq_sb[:, j:j + 1],
                                )

                # ---- K projection part 1: latt-independent s-chunk ----
                wk = [None, None]
                nc.vector.memset(KT[:, :, :, S:SP], 0.0)
                for oh in range(2):
                    wk[oh] = wpool.tile([128, 8, 512], F32R, tag="w", name=f"wk{oh}")
                    nc.sync.dma_start(wk[oh], wk_re[:, :, oh * 512:(oh + 1) * 512])

                def k_chunk(oh, s0, s1):
                    w = s1 - s0
                    for b in range(BPC):
                        for j4 in range(4):
                            j = oh * 4 + j4
                            ps = pp.tile([128, 512], F32, tag="pp", name="psk")
                            for k in range(8):
                                nc.tensor.matmul(
                                    ps[:, :w],
                                    wk[oh][:, k, j4 * 128:(j4 + 1) * 128],
                                    kv_sb[:, k, b, s0:s1],
                                    start=(k == 0), stop=(k == 7),
                                )
                            nc.scalar.activation(
                                KT[:, j, b, s0:s1], ps[:, :w],
                                Identity, bias=bk_sb[:, j:j + 1],
                            )

                k_chunk(0, *KC_HI)
                k_chunk(1, *KC_HI)

                # ---- latt = latt_raw @ Wp.T (+bp) into kv columns 0:64 ----
                with tc.tile_pool(name="lat", bufs=1) as lat:
                    lr_sb = lat.tile([128, 32, BPC * L], BF16, tag="lr")
                    nc.sync.dma_start(
                        lr_sb, lrT_d[:, :].rearrange("(k p) l -> p k l", p=128)
                    )
                    latn = lat.tile([128, D], BF16, tag="latn")
                    for oc in range(4):
                        ps = pp.tile([128, 512], F32, tag="pp")
                        for kh in range(2):
                            wp = wpool.tile([128, 16, 256], BF16, tag="wp")
                            nc.sync.dma_start(
                                wp, wp_re[:, kh * 16:(kh + 1) * 16,
                                          oc * 256:(oc + 1) * 256]
                            )
                            for k in range(16):
                                nc.tensor.matmul(
                                    ps[:, :256],
                                    lr_sb[:, kh * 16 + k, :],
                                    wp[:, k, :],
                                    start=(kh == 0 and k == 0),
                                    stop=(kh == 1 and k == 15),
                                )
                        nc.vector.tensor_copy(
                            latn[:, oc * 256:(oc + 1) * 256], ps[:, :256]
                        )
                    for j in range(8):
                        pt = pp.tile([128, 128], BF16, tag="ptr")
                        nc.tensor.transpose(
                            pt, latn[:, j * 128:(j + 1) * 128], ident
                        )
                        for b in range(BPC):
                            nc.vector.tensor_scalar_add(
                                kv_sb[:, j, b, 0:L],
                                pt[:, b * L:(b + 1) * L],
                                bp_sb[:, j:j + 1],
                            )

                # ---- K projection part 2: chunk that needs latt ----
                k_chunk(0, *KC_LO)
                k_chunk(1, *KC_LO)

                # ---- V projection: natural [s, o], 65-stride head blocks ----
                nc.vector.memset(V[:, 5, :, :], 0.0)
                wv = [None, None]
                for oh in range(2):
                    wv[oh] = wpool.tile([128, 8, 512], F32R, tag="w", name=f"wv{oh}")
                    nc.sync.dma_start(wv[oh], wv_re[:, :, oh * 512:(oh + 1) * 512])
                for oh in range(2):
                    for b in range(BPC):
                        for sc in (1, 2, 3, 4, 0, 5):
                            m = 128 if sc < 5 else 1
                            ps = pp.tile([128, 512], F32, tag="pp")
                            for k in range(8):
                                nc.tensor.matmul(
                                    ps[:m, :],
                                    kv_sb[:, k, b, sc * 128:sc * 128 + m],
                                    wv[oh][:, k, :],
                                    start=(k == 0), stop=(k == 7),
                                )
                            vv = V[:, sc, b, :].rearrange("p (h c) -> p h c", c=65)
                            nc.vector.tensor_copy(
                                vv[:m, oh * 8:(oh + 1) * 8, 0:64],
                                ps[:m, :].rearrange("p (h c) -> p h c", c=64),
                            )
                vv_all = V[:, :, :, :].rearrange("p s b (h c) -> p s b h c", c=65)
                nc.sync.dma_start(vv_all[:, :, :, :, 64], onesv_d[:, :, :, :])

            # ---- attention ----
            with (
                tc.tile_pool(name="att", bufs=1) as att,
                tc.tile_pool(name="expp", bufs=4) as expp,
                tc.tile_pool(name="zp", bufs=3) as zp,
                tc.tile_pool(name="zdp", bufs=2, space="DRAM") as zdp,
                tc.tile_pool(name="osb", bufs=3) as osbp,
            ):
                wo = att.tile([128, 8, D], F32R, tag="wo")
                nc.sync.dma_start(wo, WoT_d[:, :].rearrange("(k p) o -> p k o", p=128))
                if not zero_bias:
                    ones1_f = att.tile([1, 128], F32, tag="ones1f")
                    nc.vector.memset(ones1_f, 1.0)
                    ones1 = att.tile([1, 128], F32R, tag="ones1")
                    nc.vector.tensor_copy(ones1, ones1_f)
                    bo_sb = att.tile([1, D], F32R, tag="bo")
                    nc.sync.dma_start(bo_sb, bo_d[:, :])

                import contextlib
                _stk = contextlib.ExitStack()
                ppa = _stk.enter_context(tc.tile_pool(name="pa", bufs=2, space="PSUM"))
                ppb = _stk.enter_context(tc.tile_pool(name="pb", bufs=2, space="PSUM"))
                ppv = _stk.enter_context(tc.tile_pool(name="pv", bufs=2, space="PSUM"))

                for jp in range(8):
                    for b in range(BPC):
                        # head pair (2jp, 2jp+1): even head on PE rows 0-63,
                        # odd head on rows 64-127 -> adjacent matmuls overlap
                        ea2, pbv2, pv02, pv12 = [], [], [], []
                        for g in range(3):
                            pa2 = []
                            for hh in range(2):
                                hb = 64 * hh
                                if g == 0:
                                    ea2.append(expp.tile(
                                        [128, 6, T], BF16, tag="ea",
                                        name=f"ea{hh}"))
                                    pbv = ppb.tile([128, 7, 65], F32,
                                                   tag="pbv", name=f"pbv{hh}")
                                    pbv2.append(pbv)
                                pa2.append(ppa.tile(
                                    [128, 2, 512], F32, tag="pa",
                                    name=f"pa{hh}"))
                            for sc2 in range(2):
                                sc = g * 2 + sc2
                                for hh in range(2):
                                    hb = 64 * hh
                                    kt = KT[hb:hb + 64, jp, b,
                                            sc * 128:(sc + 1) * 128]
                                    nc.tensor.matmul(
                                        pa2[hh][:, sc2, :], kt,
                                        QT[hb:hb + 64, jp, b, 0:512],
                                        start=True, stop=True,
                                    )
                                for hh in range(2):
                                    hb = 64 * hh
                                    kt = KT[hb:hb + 64, jp, b,
                                            sc * 128:(sc + 1) * 128]
                                    nc.tensor.matmul(
                                        pbv2[hh][:, sc, :], kt,
                                        QT[hb:hb + 64, jp, b, 512:T],
                                        start=True, stop=True,
                                    )
                            for hh in range(2):
                                nc.scalar.activation(
                                    ea2[hh][:, g * 2:(g + 1) * 2, 0:512],
                                    pa2[hh], Exp, bias=0.0, scale=SCALE,
                                )
                        for hh in range(2):
                            nc.scalar.activation(
                                ea2[hh][:, :, 512:T], pbv2[hh][:, 0:6, :],
                                Exp, bias=0.0, scale=SCALE,
                            )

                        for hh in range(2):
                            h, hb = 2 * jp + hh, 64 * hh
                            j = jp
                            ea = ea2[hh]
                            pv0 = ppv.tile([65, 512], F32, tag="pv0",
                                           name=f"pv0{hh}")
                            pv1 = pbv2[hh][0:65, 6, :]
                            for sc in range(6):
                                vh = V[:, sc, b, 65 * h:65 * h + 65]
                                nc.tensor.matmul(pv0, vh, ea[:, sc, 0:512],
                                                 start=(sc == 0), stop=(sc == 5))
                                nc.tensor.matmul(pv1, vh, ea[:, sc, 512:T],
                                                 start=(sc == 0), stop=(sc == 5))

                            # evacuate PV psum fast; broadcast 1/Z across
                            # partitions via DRAM-bounce stride-0 DMA
                            zs = zp.tile([65, T], F32, tag="zs")
                            nc.vector.reciprocal(zs[64:65, 0:512], pv0[64:65, :])
                            nc.vector.reciprocal(zs[64:65, 512:T], pv1[64:65, :])
                            pvs = zp.tile([64, T], F32, tag="pvs")
                            nc.vector.tensor_copy(pvs[:, 0:512], pv0[0:64, :])
                            nc.vector.tensor_copy(pvs[:, 512:T], pv1[0:64, :])
                            zb = zp.tile([64, T], F32, tag="zb")
                            zd = zdp.tile([1, T], F32, tag="zd")
                            nc.gpsimd.dma_start(zd, zs[64:65, :])
                            zdsrc = zd[0:1, :]
                            src = bass.AP(
                                tensor=zdsrc.tensor, offset=zdsrc.offset,
                                ap=[[0, 64]] + [list(d) for d in zdsrc.ap[1:]],
                            )
                            nc.gpsimd.dma_start(zb, src)
                            nc.vector.tensor_tensor(
                                ctxT[hb:hb + 64, j, b, 0:512],
                                pvs[:, 0:512], zb[:, 0:512], MULT,
                            )
                            nc.vector.tensor_tensor(
                                ctxT[hb:hb + 64, j, b, 512:T],
                                pvs[:, 512:T], zb[:, 512:T], MULT,
                            )
                            if not zero_bias:
                                for (t0, t1) in ((0, 512), (512, T)):
                                    nc.vector.tensor_scalar_add(
                                        ctxT[hb:hb + 64, j, b, t0:t1],
                                        ctxT[hb:hb + 64, j, b, t0:t1],
                                        bv_sb[hb:hb + 64, j:j + 1],
                                    )

                _stk.close()

                # ---- output projection: out[t, o] = ctxT.T @ WoT (+bo) ----
                import contextlib as _ctxlib
                _stk2 = _ctxlib.ExitStack()
                ppo = _stk2.enter_context(
                    tc.tile_pool(name="ppo", bufs=2, space="PSUM"))
                for b in range(BPC):
                    for tcn in range(5):
                        t0 = tcn * 128
                        m = min(128, T - t0)
                        osb = osbp.tile([128, D], F32, tag="osb")
                        for oc in range(2):
                            ps = ppo.tile([128, 512], F32, tag="ppo")
                            for k in range(8):
                                nc.tensor.matmul(
                                    ps[:m, :],
                                    ctxT[:, k, b, t0:t0 + m],
                                    wo[:, k, oc * 512:(oc + 1) * 512],
                                    start=(k == 0), stop=(zero_bias and k == 7),
                                )
                            if not zero_bias:
                                nc.tensor.matmul(
                                    ps[:m, :], ones1[0:1, :m],
                                    bo_sb[0:1, oc * 512:(oc + 1) * 512],
                                    start=False, stop=True,
                                )
                            nc.scalar.copy(
                                osb[:m, oc * 512:(oc + 1) * 512], ps[:m, :]
                            )
                        nc.sync.dma_start(out_d[b, t0:t0 + m, :], osb[:m, :])
                _stk2.close()

    nc.finalize()
    return nc


_NC_CACHE = {}
LAST_RESULT = None


def kernel(hidden_states, latt_raw, Wp, bp, Wq, bq, Wk, bk, Wv, bv, Wo, bo,
           trace=False):
    global LAST_RESULT
    f = lambda x: np.ascontiguousarray(np.asarray(x), dtype=np.float32)
    hs, lr = f(hidden_states), f(latt_raw)
    Wp, Wq, Wk, Wv, Wo = f(Wp), f(Wq), f(Wk), f(Wv), f(Wo)
    bp, bq, bk, bv, bo = f(bp), f(bq), f(bk), f(bv), f(bo)

    zero_bias = not any(x.any() for x in (bp, bq, bk, bv, bo))
    key = zero_bias
    if key not in _NC_CACHE:
        _NC_CACHE[key] = build_nc(zero_bias)
    nc = _NC_CACHE[key]

    WpT = np.ascontiguousarray(Wp.T.astype(ml_dtypes.bfloat16))
    WqT = np.ascontiguousarray(Wq.T)
    WkT = np.ascontiguousarray(Wk.T)
    WvT = np.ascontiguousarray(Wv.T)
    WoT = np.ascontiguousarray(Wo.T)
    b2 = lambda x: np.ascontiguousarray(x.reshape(8, 128).T)
    bq2, bk2, bp2, bv2 = b2(bq), b2(bk), b2(bp), b2(bv)
    bo2 = np.ascontiguousarray(bo[None, :])

    p = np.arange(128)[:, None]
    sc = np.arange(6)[None, :]
    valid = (sc * 128 + p) < S                       # [128, 6]
    onesv = np.broadcast_to(
        valid[:, :, None, None], (128, 6, BPC, H)
    ).astype(ml_dtypes.bfloat16)
    onesv = np.ascontiguousarray(onesv)

    in_maps = []
    for c in range(NC):
        hsb = hs[c * BPC:(c + 1) * BPC]              # [2, 577, 1024]
        kvt = np.zeros((BPC, D, TQ), np.float32)
        kvt[:, :, 0:T] = hsb.transpose(0, 2, 1)
        lrt = np.concatenate(
            [lr[c * BPC + b].T for b in range(BPC)], axis=1
        ).astype(ml_dtypes.bfloat16)                  # [4096, 128]
        in_maps.append({
            "kvT": kvt, "lrT": np.ascontiguousarray(lrt),
            "WpT": WpT, "WqT": WqT, "WkT": WkT, "WvT": WvT, "WoT": WoT,
            "bq2": bq2, "bk2": bk2, "bp2": bp2, "bv2": bv2, "bo2": bo2,
            "onesv": onesv,
        })

    LAST_RESULT = run_bass_kernel_spmd(
        nc, in_maps, core_ids=list(range(NC)), trace=trace
    )
    outs = [r["out2"] for r in LAST_RESULT.results]
    return np.ascontiguousarray(np.concatenate(outs, axis=0), dtype=np.float32)


# revision 43
# speedup vs baseline: 1.0004x; 1.0004x over previous
"""Trainium2 Bass kernel for nn_CLIPVisionTower (latent-token attention block).

Strategy: data-parallel over batch (16 batches -> 8 cores x 2). Each core runs the
full block for its 2 batch elements; host concatenates outputs. No collectives.

Layout notes:
- All weights are passed host-transposed ([d_in, d_out]) and declared as float32r
  DRAM (raw fp32 bytes; the PE f32r path runs 1 cycle/row at free-dim >= 256).
- Activations flow "transposed" (feature dim on SBUF partitions): kvT holds
  [latt | hidden]^T per batch. Keys are zero-padded to 768 rows so every
  attention s-chunk is a full 128-row matmul; padded keys contribute exp(0)=1
  against V rows that are exactly zero and a ones-column that the host zeroes,
  so no masking instructions are needed.
- Softmax skips max-subtraction (logits*0.125 is O(5); exp is safe in fp32) and
  runs in key-major layout: exp on ScalarE straight out of PSUM; the Z row rides
  along as a 65th ones-column of V; 1/Z is computed by DVE reciprocal straight
  from PSUM and broadcast across partitions with a partition-stride-0 SWDGE DMA.
- Attention tensors (Q^T, K^T, V, exp(probs)) are bf16; projections are f32r.
"""

import sys

sys.path.insert(0, "/opt/trn_rl_repo")

import numpy as np
import ml_dtypes

import concourse.bass as bass
import concourse.mybir as mybir
import concourse.tile as tile
from concourse import bacc
from concourse.bass_utils import run_bass_kernel_spmd
from concourse.masks import make_identity

B, T, D = 16, 577, 1024
L, D_LLM = 64, 4096
H, HD = 16, 64
SCALE = HD ** -0.5
S = L + T            # 641 kv rows
SP = 768             # padded key rows for attention (6 * 128)
NC = 8               # cores
BPC = B // NC        # batches per core = 2

F32 = mybir.dt.float32
F32R = mybir.dt.float32r
BF16 = mybir.dt.bfloat16
Exp = mybir.ActivationFunctionType.Exp
Identity = mybir.ActivationFunctionType.Identity
MULT = mybir.AluOpType.mult

# f32r matmuls need even output widths
TQ = 578             # padded token axis for QT
SQ = 642             # padded kv column count (even)
TC2 = [(0, 290), (290, 578)]          # Q-proj psum chunks
KC_HI = (384, 642)                    # K-proj s-chunk independent of latt
KC_LO = (0, 384)                      # K-proj s-chunk that needs latt


def build_nc(zero_bias: bool):
    nc = bacc.Bacc(None, target_bir_lowering=False)

    kvT_d = nc.dram_tensor("kvT", [BPC, D, TQ], F32R, kind="ExternalInput")
    lrT_d = nc.dram_tensor("lrT", [D_LLM, BPC * L], BF16, kind="ExternalInput")
    WpT_d = nc.dram_tensor("WpT", [D_LLM, D], BF16, kind="ExternalInput")
    WqT_d = nc.dram_tensor("WqT", [D, D], F32R, kind="ExternalInput")
    WkT_d = nc.dram_tensor("WkT", [D, D], F32R, kind="ExternalInput")
    WvT_d = nc.dram_tensor("WvT", [D, D], F32R, kind="ExternalInput")
    WoT_d = nc.dram_tensor("WoT", [D, D], F32R, kind="ExternalInput")
    bq_d = nc.dram_tensor("bq2", [128, 8], F32, kind="ExternalInput")
    bk_d = nc.dram_tensor("bk2", [128, 8], F32, kind="ExternalInput")
    bp_d = nc.dram_tensor("bp2", [128, 8], F32, kind="ExternalInput")
    bv_d = nc.dram_tensor("bv2", [128, 8], F32, kind="ExternalInput")
    bo_d = nc.dram_tensor("bo2", [1, D], F32R, kind="ExternalInput")
    onesv_d = nc.dram_tensor("onesv", [128, 6, BPC, H], BF16, kind="ExternalInput")
    out_d = nc.dram_tensor("out2", [BPC, T, D], F32, kind="ExternalOutput")

    with tile.TileContext(nc) as tc:
        with tc.tile_pool(name="big", bufs=1) as big:
            # ---- persistent tensors ----
            QT = big.tile([128, 8, BPC, TQ], BF16, tag="qt")
            KT = big.tile([128, 8, BPC, SP], BF16, tag="kt")
            V = big.tile([128, 6, BPC, H * 65], BF16, tag="v")
            ctxT = big.tile([128, 8, BPC, T], F32R, tag="ctx")

            ident = big.tile([128, 128], BF16, tag="ident")
            bq_sb = big.tile([128, 8], F32, tag="bq")
            bk_sb = big.tile([128, 8], F32, tag="bk")
            bp_sb = big.tile([128, 8], F32, tag="bp")
            bv_sb = big.tile([128, 8], F32, tag="bv")

            with (
                tc.tile_pool(name="kvpool", bufs=1) as kvpool,
                tc.tile_pool(name="wpool", bufs=2) as wpool,
                tc.tile_pool(name="pp", bufs=3, space="PSUM") as pp,
            ):
                kv_sb = kvpool.tile([128, 8, BPC, SQ], F32R, tag="kv")
                kv_re = kvT_d[:, :, :].rearrange("b (j p) s -> p j b s", p=128)
                wq_re = WqT_d[:, :].rearrange("(k p) o -> p k o", p=128)
                wk_re = WkT_d[:, :].rearrange("(k p) o -> p k o", p=128)
                wv_re = WvT_d[:, :].rearrange("(k p) o -> p k o", p=128)
                wp_re = WpT_d[:, :].rearrange("(k p) o -> p k o", p=128)

                # ---- Q projection (emitted first: smallest DMA prefix) ----
                wq = [None, None]
                for oh in range(2):
                    wq[oh] = wpool.tile([128, 8, 512], F32R, tag="w", name=f"wq{oh}")
                    nc.sync.dma_start(wq[oh], wq_re[:, :, oh * 512:(oh + 1) * 512])
                    if oh == 0:
                        for b in range(BPC):
                            nc.sync.dma_start(
                                kv_sb[:, :, b, L:SQ], kv_re[:, :, b, :])
                        # consts ride behind the critical loads
                        make_identity(nc, ident)
                        nc.sync.dma_start(bq_sb, bq_d[:, :])
                        nc.sync.dma_start(bk_sb, bk_d[:, :])
                        nc.sync.dma_start(bp_sb, bp_d[:, :])
                        nc.sync.dma_start(bv_sb, bv_d[:, :])
                    for b in range(BPC):
                        for j4 in range(4):
                            j = oh * 4 + j4
                            for (t0, t1) in TC2:
                                w = t1 - t0
                                ps = pp.tile([128, 512], F32, tag="pp")
                                for k in range(8):
                                    nc.tensor.matmul(
                                        ps[:, :w],
                                        wq[oh][:, k, j4 * 128:(j4 + 1) * 128],
                                        kv_sb[:, k, b, L + t0:L + t1],
                                        start=(k == 0), stop=(k == 7),
                                    )
                                nc.scalar.activation(
                                    QT[:, j, b, t0:t1], ps[:, :w],
                                    Identity, bias=bq_sb[:, j:j + 1],
                                )

                # ---- K projection part 1: latt-independent s-chunk ----
                wk = [None, None]
                nc.vector.memset(KT[:, :, :, S:SP], 0.0)
                for oh in range(2):
                    wk[oh] = wpool.tile([128, 8, 512], F32R, tag="w", name=f"wk{oh}")
                    nc.sync.dma_start(wk[oh], wk_re[:, :, oh * 512:(oh + 1) * 512])

                def k_chunk(oh, s0, s1):
                    w = s1 - s0
                    for b in range(BPC):
                        for j4 in range(4):
                            j = oh * 4 + j4
                            ps = pp.tile([128, 512], F32, tag="pp", name="psk")
                            for k in range(8):
                                nc.tensor.matmul(
                                    ps[:, :w],
                                    wk[oh][:, k, j4 * 128:(j4 + 1) * 128],
                                    kv_sb[:, k, b, s0:s1],
                                    start=(k == 0), stop=(k == 7),
                                )
                            nc.scalar.activation(
                                KT[:, j, b, s0:s1], ps[:, :w],
                                Identity, bias=bk_sb[:, j:j + 1],
                            )

                k_chunk(0, *KC_HI)
                k_chunk(1, *KC_HI)

                # ---- latt = latt_raw @ Wp.T (+bp) into kv columns 0:64 ----
                with tc.tile_pool(name="lat", bufs=1) as lat:
                    lr_sb = lat.tile([128, 32, BPC * L], BF16, tag="lr")
                    nc.sync.dma_start(
                        lr_sb, lrT_d[:, :].rearrange("(k p) l -> p k l", p=128)
                    )
                    latn = lat.tile([128, D], BF16, tag="latn")
                    for oc in range(4):
                        ps = pp.tile([128, 512], F32, tag="pp")
                        for kh in range(2):
                            wp = wpool.tile([128, 16, 256], BF16, tag="wp")
                            nc.sync.dma_start(
                                wp, wp_re[:, kh * 16:(kh + 1) * 16,
                                          oc * 256:(oc + 1) * 256]
                            )
                            for k in range(16):
                                nc.tensor.matmul(
                                    ps[:, :256],
                                    lr_sb[:, kh * 16 + k, :],
                                    wp[:, k, :],
                                    start=(kh == 0 and k == 0),
                                    stop=(kh == 1 and k == 15),
                                )
                        nc.vector.tensor_copy(
                            latn[:, oc * 256:(oc + 1) * 256], ps[:, :256]
                        )
                    for j in range(8):
                        pt = pp.tile([128, 128], BF16, tag="ptr")
                        nc.tensor.transpose(
                            pt, latn[:, j * 128:(j + 1) * 128], ident
                        )
                        for b in range(BPC):
                            nc.vector.tensor_scalar_add(
                                kv_sb[:, j, b, 0:L],
                                pt[:, b * L:(b + 1) * L],
                                bp_sb[:, j:j + 1],
                            )

                # ---- K projection part 2: chunk that needs latt ----
                k_chunk(0, *KC_LO)
                k_chunk(1, *KC_LO)

                # ---- V projection: natural [s, o], 65-stride head blocks ----
                nc.vector.memset(V[:, 5, :, :], 0.0)
                wv = [None, None]
                for oh in range(2):
                    wv[oh] = wpool.tile([128, 8, 512], F32R, tag="w", name=f"wv{oh}")
                    nc.sync.dma_start(wv[oh], wv_re[:, :, oh * 512:(oh + 1) * 512])
                for oh in range(2):
                    for b in range(BPC):
                        for sc in (1, 2, 3, 4, 0, 5):
                            m = 128 if sc < 5 else 1
                            ps = pp.tile([128, 512], F32, tag="pp")
                            for k in range(8):
                                nc.tensor.matmul(
                                    ps[:m, :],
                                    kv_sb[:, k, b, sc * 128:sc * 128 + m],
                                    wv[oh][:, k, :],
                                    start=(k == 0), stop=(k == 7),
                                )
                            vv = V[:, sc, b, :].rearrange("p (h c) -> p h c", c=65)
                            nc.vector.tensor_copy(
                                vv[:m, oh * 8:(oh + 1) * 8, 0:64],
                                ps[:m, :].rearrange("p (h c) -> p h c", c=64),
                            )
                vv_all = V[:, :, :, :].rearrange("p s b (h c) -> p s b h c", c=65)
                nc.sync.dma_start(vv_all[:, :, :, :, 64], onesv_d[:, :, :, :])

            # ---- attention ----
            with (
                tc.tile_pool(name="att", bufs=1) as att,
                tc.tile_pool(name="expp", bufs=4) as expp,
                tc.tile_pool(name="zp", bufs=3) as zp,
                tc.tile_pool(name="zdp", bufs=6, space="DRAM") as zdp,
                tc.tile_pool(name="osb", bufs=3) as osbp,
            ):
                wo = att.tile([128, 8, D], F32R, tag="wo")
                nc.sync.dma_start(wo, WoT_d[:, :].rearrange("(k p) o -> p k o", p=128))
                if not zero_bias:
                    ones1_f = att.tile([1, 128], F32, tag="ones1f")
                    nc.vector.memset(ones1_f, 1.0)
                    ones1 = att.tile([1, 128], F32R, tag="ones1")
                    nc.vector.tensor_copy(ones1, ones1_f)
                    bo_sb = att.tile([1, D], F32R, tag="bo")
                    nc.sync.dma_start(bo_sb, bo_d[:, :])

                import contextlib
                _stk = contextlib.ExitStack()
                ppa = _stk.enter_context(tc.tile_pool(name="pa", bufs=2, space="PSUM"))
                ppb = _stk.enter_context(tc.tile_pool(name="pb", bufs=2, space="PSUM"))
                ppv = _stk.enter_context(tc.tile_pool(name="pv", bufs=2, space="PSUM"))

                for jp in range(8):
                    for b in range(BPC):
                        # head pair (2jp, 2jp+1): even head on PE rows 0-63,
                        # odd head on rows 64-127 -> adjacent matmuls overlap
                        ea2, pbv2, pv02, pv12 = [], [], [], []
                        for g in range(3):
                            pa2 = []
                            for hh in range(2):
                                hb = 64 * hh
                                if g == 0:
                                    ea2.append(expp.tile(
                                        [128, 6, T], BF16, tag="ea",
                                        name=f"ea{hh}"))
                                    pbv = ppb.tile([128, 7, 65], F32,
                                                   tag="pbv", name=f"pbv{hh}")
                                    pbv2.append(pbv)
                                pa2.append(ppa.tile(
                                    [128, 2, 512], F32, tag="pa",
                                    name=f"pa{hh}"))
                            for sc2 in range(2):
                                sc = g * 2 + sc2
                                for hh in range(2):
                                    hb = 64 * hh
                                    kt = KT[hb:hb + 64, jp, b,
                                            sc * 128:(sc + 1) * 128]
                                    nc.tensor.matmul(
                                        pa2[hh][:, sc2, :], kt,
                                        QT[hb:hb + 64, jp, b, 0:512],
                                        start=True, stop=True,
                                    )
                                for hh in range(2):
                                    hb = 64 * hh
                                    kt = KT[hb:hb + 64, jp, b,
                                            sc * 128:(sc + 1) * 128]
                                    nc.tensor.matmul(
                                        pbv2[hh][:, sc, :], kt,
                                        QT[hb:hb + 64, jp, b, 512:T],
                                        start=True, stop=True,
                                    )
                            for hh in range(2):
                                nc.scalar.activation(
                                    ea2[hh][:, g * 2:(g + 1) * 2, 0:512],
                                    pa2[hh], Exp, bias=0.0, scale=SCALE,
                                )
                        for hh in range(2):
                            nc.scalar.activation(
                                ea2[hh][:, :, 512:T], pbv2[hh][:, 0:6, :],
                                Exp, bias=0.0, scale=SCALE,
                            )

                        for hh in range(2):
                            h, hb = 2 * jp + hh, 64 * hh
                            j = jp
                            ea = ea2[hh]
                            pv0 = ppv.tile([65, 512], F32, tag="pv0",
                                           name=f"pv0{hh}")
                            pv1 = pbv2[hh][0:65, 6, :]
                            for sc in range(6):
                                vh = V[:, sc, b, 65 * h:65 * h + 65]
                                nc.tensor.matmul(pv0, vh, ea[:, sc, 0:512],
                                                 start=(sc == 0), stop=(sc == 5))
                                nc.tensor.matmul(pv1, vh, ea[:, sc, 512:T],
                                                 start=(sc == 0), stop=(sc == 5))

                            # evacuate PV psum fast; broadcast 1/Z across
                            # partitions via DRAM-bounce stride-0 DMA
                            zs = zp.tile([65, T], F32, tag="zs")
                            nc.vector.reciprocal(zs[64:65, 0:512], pv0[64:65, :])
                            nc.vector.reciprocal(zs[64:65, 512:T], pv1[64:65, :])
                            pvs = zp.tile([64, T], F32, tag="pvs")
                            nc.vector.tensor_copy(pvs[:, 0:512], pv0[0:64, :])
                            nc.vector.tensor_copy(pvs[:, 512:T], pv1[0:64, :])
                            zb = zp.tile([64, T], F32, tag="zb")
                            zd = zdp.tile([1, T], F32, tag="zd")
                            nc.gpsimd.dma_start(zd, zs[64:65, :])
                            zdsrc = zd[0:1, :]
                            src = bass.AP(
                                tensor=zdsrc.tensor, offset=zdsrc.offset,
                                ap=[[0, 64]] + [list(d) for d in zdsrc.ap[1:]],
                            )
                            nc.gpsimd.dma_start(zb, src)
                            nc.vector.tensor_tensor(
                                ctxT[hb:hb + 64, j, b, 0:512],
                                pvs[:, 0:512], zb[:, 0:512], MULT,
                            )
                            nc.vector.tensor_tensor(
                                ctxT[hb:hb + 64, j, b, 512:T],
                                pvs[:, 512:T], zb[:, 512:T], MULT,
                            )
                            if not zero_bias:
                                for (t0, t1) in ((0, 512), (512, T)):
                                    nc.vector.tensor_scalar_add(
                                        ctxT[hb:hb + 64, j, b, t0:t1],
                                        ctxT[hb:hb + 64, j, b, t0:t1],
                                        bv_sb[hb:hb + 64, j:j + 1],
                                    )

                _stk.close()

                # ---- output projection: out[t, o] = ctxT.T @ WoT (+bo) ----
                import contextlib as _ctxlib
                _stk2 = _ctxlib.ExitStack()
                ppo = _stk2.enter_context(
                    tc.tile_pool(name="ppo", bufs=4, space="PSUM"))
                for b in range(BPC):
                    for tcn in range(5):
                        t0 = tcn * 128
                        m = min(128, T - t0)
                        osb = osbp.tile([128, D], F32, tag="osb")
                        for oc in range(2):
                            ps = ppo.tile([128, 512], F32, tag="ppo")
                            for k in range(8):
                                nc.tensor.matmul(
                                    ps[:m, :],
                                    ctxT[:, k, b, t0:t0 + m],
                                    wo[:, k, oc * 512:(oc + 1) * 512],
                                    start=(k == 0), stop=(zero_bias and k == 7),
                                )
                            if not zero_bias:
                                nc.tensor.matmul(
                                    ps[:m, :], ones1[0:1, :m],
                                    bo_sb[0:1, oc * 512:(oc + 1) * 512],
                                    start=False, stop=True,
                                )
                            nc.scalar.copy(
                                osb[:m, oc * 512:(oc + 1) * 512], ps[:m, :]
                            )
                        nc.sync.dma_start(out_d[b, t0:t0 + m, :], osb[:m, :])
                _stk2.close()

    nc.finalize()
    return nc


_NC_CACHE = {}
LAST_RESULT = None


def kernel(hidden_states, latt_raw, Wp, bp, Wq, bq, Wk, bk, Wv, bv, Wo, bo,
           trace=False):
    global LAST_RESULT
    f = lambda x: np.ascontiguousarray(np.asarray(x), dtype=np.float32)
    hs, lr = f(hidden_states), f(latt_raw)
    Wp, Wq, Wk, Wv, Wo = f(Wp), f(Wq), f(Wk), f(Wv), f(Wo)
    bp, bq, bk, bv, bo = f(bp), f(bq), f(bk), f(bv), f(bo)

    zero_bias = not any(x.any() for x in (bp, bq, bk, bv, bo))
    key = zero_bias
    if key not in _NC_CACHE:
        _NC_CACHE[key] = build_nc(zero_bias)
    nc = _NC_CACHE[key]

    WpT = np.ascontiguousarray(Wp.T.astype(ml_dtypes.bfloat16))
    WqT = np.ascontiguousarray(Wq.T)
    WkT = np.ascontiguousarray(Wk.T)
    WvT = np.ascontiguousarray(Wv.T)
    WoT = np.ascontiguousarray(Wo.T)
    b2 = lambda x: np.ascontiguousarray(x.reshape(8, 128).T)
    bq2, bk2, bp2, bv2 = b2(bq), b2(bk), b2(bp), b2(bv)
    bo2 = np.ascontiguousarray(bo[None, :])

    p = np.arange(128)[:, None]
    sc = np.arange(6)[None, :]
    valid = (sc * 128 + p) < S                       # [128, 6]
    onesv = np.broadcast_to(
        valid[:, :, None, None], (128, 6, BPC, H)
    ).astype(ml_dtypes.bfloat16)
    onesv = np.ascontiguousarray(onesv)

    in_maps = []
    for c in range(NC):
        hsb = hs[c * BPC:(c + 1) * BPC]              # [2, 577, 1024]
        kvt = np.zeros((BPC, D, TQ), np.float32)
        kvt[:, :, 0:T] = hsb.transpose(0, 2, 1)
        lrt = np.concatenate(
            [lr[c * BPC + b].T for b in range(BPC)], axis=1
        ).astype(ml_dtypes.bfloat16)                  # [4096, 128]
        in_maps.append({
            "kvT": kvt, "lrT": np.ascontiguousarray(lrt),
            "WpT": WpT, "WqT": WqT, "WkT": WkT, "WvT": WvT, "WoT": WoT,
            "bq2": bq2, "bk2": bk2, "bp2": bp2, "bv2": bv2, "bo2": bo2,
            "onesv": onesv,
        })

    LAST_RESULT = run_bass_kernel_spmd(
        nc, in_maps, core_ids=list(range(NC)), trace=trace
    )
    outs = [r["out2"] for r in LAST_RESULT.results]
    return np.ascontiguousarray(np.concatenate(outs, axis=0), dtype=np.float32)
